# revision 61
# baseline (speedup 1.0000x reference)
"""Trainium2 Bass kernel for nn_ExtractorMLP: per-edge MLP over gathered node
embeddings, data-parallel over edges across 8 NeuronCores.

Per edge e: out = relu(relu(concat(emb[col[e]], emb[row[e]]) @ W1 + b1) @ W2 + b2) @ W3 + b3

v3 strategy ("host-sequenced gather, pure streaming MLP on device"):
The v1 kernel's critical path was the on-device gather: row-side indirect DMA
(784 GpSimd calls/core at ~1.2us) and col-side one-hot selection matmuls
(+2560 PE cycles/block).  v1 already shipped host-sequenced per-block chunk
data (chks/colf, ~100MB/core); v2+ pushes that to its logical end: the host
ships the gathered endpoint features directly, transposed to the [feature,
edge] layout the PE wants, interleaved per 512-edge block as crt[b] =
[colT_blk | rowT_blk] (same ~100MB/core of DRAM traffic).  The device is then
a pure streaming MLP at the PE roofline; the v2 trace showed 95.5% PE
occupancy with a 232ns MM issue period and <1us of total PE idle.

Refinements over the plain streaming version (measured 618us -> 547us):
- all-bf16 pipeline: with f32r operands every matmul paid ~20ns extra (no
  Fast Weight Load for fp32 -- LDWEIGHTS 187ns vs 97ns -- plus fp32-HIGH
  weight pairing overhead); bf16 operands hit the textbook warm issue period
  (~213ns = 512 cols @ 2.4GHz) and halve input DMA to ~50MB/core.  Measured
  596us (f32r) -> 548us (bf16); rel err 4.9e-3 vs 2e-2 gate.  fp8/DoubleRow
  (K=256 virtualized, would halve h1) was host-emulated at rel err 4.9e-2 --
  fails the gate; rejected without burning a hardware run.
- w3 packing: the [128]->[2] output matmul wastes 126/128 PE rows.  Four
  consecutive blocks' w3 matmuls are issued back-to-back into disjoint
  32-column PE strips (tile_position=(0,32j), out partitions 32j:32j+2 of one
  PSUM bank) so they execute concurrently (~630ns per 4 blocks incl. the two
  tiling-mode-switch drains, vs 4x213ns unpacked), and a single [128,512]
  ACTIVATE evacuates all four (ACT cost is free-dim-based).  Host unpacks
  partition strips.  fp32 operands cannot column-tile (ISA dst-partition
  check); larger groups (7 waves per mode switch) measured WORSE (652us):
  wave N+3's PSUM bank depends on wave N's evacuation, which queues behind
  the regular relus in the ACT/DVE FIFOs.
- startup/tail: crt[0]+w1 DMAs issued first (first MM at ~10.5us); output
  DMAs ride the ScalarE HWDGE ring so cr prefetches never queue behind them
  on the Sync ring.
Final trace accounting: MM span 531us = 2352 MMs x 213ns + 49 w3 groups x
~630ns (within ~1us of the streaming-roofline model), plus ~10.5us fixed
startup and ~5.3us tail.  Measured 547us HW exec at full clock (the part
sometimes power-throttles to ~2.0GHz under sustained load -> ~650-715us);
rel err 4.9e-3.

Software pipelining keeps every engine's inputs at least one full block ahead
of use (PE never waits on relu evacuation): iteration i runs h1 pairs of
block i, h2 of block i-1, and the packed w3 group g=(i-5)/4 covering blocks
4g..4g+3.  PSUM: h1 m-groups rotate over 4 banks, h2 over 2, w3-out over 2.
Relu+bias evacuation is split between ScalarE (h1 m0/m1, out) and VectorE
(h1 m2/m3, h2; fused add-bias+max-0 tensor_scalar).  All matmuls in float32r
(TF32-like, ~3e-4 rel err, full PE rate); f32r DRAM tensors are DMAed
straight into f32r SBUF tiles (f32r is bit-identical to f32).  No sort, no
permutation: edges keep their natural order."""

import sys

import numpy as np

N_NODES = 50000
HIDDEN = 128
N_EDGES = 800000
N_CORES = 8
E_SHARD = N_EDGES // N_CORES

BLOCK = 512
N_BLOCKS = 196
E_PAD = N_BLOCKS * BLOCK   # 100352
WGRP = 4                   # blocks per packed w3 group (4 col-tiled strips)
N_GRPS = N_BLOCKS // WGRP  # 49

_REPO = "/opt/trn_rl_repo"
_prog_cache = {}
RUN_KWARGS = {}
LAST_RESULTS = None


def _build_program(n_blocks=N_BLOCKS, debug=False):
    if _REPO not in sys.path:
        sys.path.insert(0, _REPO)
    from concourse import bacc, mybir
    import concourse.tile as tile

    f32 = mybir.dt.float32
    f32r = mybir.dt.float32r
    bf16 = mybir.dt.bfloat16
    Relu = mybir.ActivationFunctionType.Relu
    Ident = mybir.ActivationFunctionType.Identity
    ADD = mybir.AluOpType.add
    MAX = mybir.AluOpType.max

    n_grps = n_blocks // WGRP

    nc = bacc.Bacc("TRN2", target_bir_lowering=False, debug=debug)
    # per-block gathered features: crt[b][:, 0:512] = emb[col].T for the
    # block's 512 edges, crt[b][:, 512:1024] = emb[row].T
    fp8 = mybir.dt.float8e4
    DR = mybir.MatmulPerfMode.DoubleRow
    # per-block gathered features in fp8 hi/lo form, slot order
    # [xhc, xlc, xhr, xlr, xhc/S, xhr/S] so the three DoubleRow term-pairs
    # (0,1) (2,3) (4,5) are adjacent slot slices.  The lo-weights are scaled
    # by S=32 into e4m3's normal range (unscaled they are subnormal and the
    # correction quantizes to garbage); the paired rhs uses xh/S (an exact
    # exponent shift) so the product keeps scale 1 and accumulates into the
    # same PSUM group.  Dropped wl*xl terms are O(eps^2).
    crt = nc.dram_tensor(
        "crt", [n_blocks, 128, 6, BLOCK], fp8, kind="ExternalInput"
    )
    # w1 fp8 DoubleRow packs: kw8[m*3+p] = [128, 2, 128] weight pair for
    # h1 m-group term-pair p: (whc,whc), (whr,whr), (wlc*S, wlr*S)
    kw8 = nc.dram_tensor("kw8", [128, 12, 2, 128], fp8, kind="ExternalInput")
    # packed constants: kw = [w2 (512) | w3 (2)] bf16, kb = biases f32
    kw = nc.dram_tensor("kw", [128, 514], bf16, kind="ExternalInput")
    kb = nc.dram_tensor("kb", [128, 6], f32, kind="ExternalInput")
    # packed output: group g holds blocks 4g..4g+3 at partitions 32j:32j+2
    out_t = nc.dram_tensor("out_t", [n_grps, 128, BLOCK], f32, kind="ExternalOutput")

    with tile.TileContext(nc) as tc:
        with (
            tc.tile_pool(name="const", bufs=1) as cp,
            tc.tile_pool(name="inp", bufs=6) as inp,
            tc.tile_pool(name="h1", bufs=3) as h1pool,
            tc.tile_pool(name="h2", bufs=8) as h2pool,
            tc.tile_pool(name="oac", bufs=3) as opool,
            tc.tile_pool(name="ps_h1", bufs=4, space="PSUM") as ph1,
            tc.tile_pool(name="ps_h2", bufs=2, space="PSUM") as ph2,
            tc.tile_pool(name="ps_o", bufs=2, space="PSUM") as po,
        ):
            # ---- persistent constants ----
            # crt[0] and the w1 fp8 packs first: the startup-critical first
            # h1 matmuls gate only on these two transfers
            cr0 = inp.tile([128, 6, BLOCK], fp8, tag="cr")
            nc.sync.dma_start(out=cr0[:], in_=crt[0])
            kw8_sb = cp.tile([128, 12, 2, 128], fp8)
            nc.sync.dma_start(out=kw8_sb[:], in_=kw8[:])
            kw_sb = cp.tile([128, 514], bf16)
            nc.sync.dma_start(out=kw_sb[:], in_=kw[:])
            kb_sb = cp.tile([128, 6], f32)
            nc.sync.dma_start(out=kb_sb[:], in_=kb[:])
            w2_sb = kw_sb[:, 0:512]
            b1_sb = kb_sb[:, 0:4]
            b2_sb = kb_sb[:, 4:5]
            b3_sb = kb_sb[:, 5:6]
            w3_bf = kw_sb[:, 512:514]

            # w3 group schedule: full 8-block groups two iterations after the
            # group's last h2 stage; the 4-block tail group at the very end
            w3_at = {}
            for g in range(n_blocks // WGRP):
                w3_at[WGRP * g + WGRP + 2] = g
            if n_blocks % WGRP:
                w3_at[n_blocks + 2] = n_blocks // WGRP

            h1T_hist = {}   # block id -> h1T tile (consumed by h2 one iter later)
            h2T_hist = {}   # pair id -> paired h2T tile (consumed by w3 group)
            last_it = max(w3_at)
            for it in range(last_it + 1):
                b = it            # h1 stage block
                bh2 = it - 1      # h2 stage block

                if b < n_blocks:
                    if b == 0:
                        cr = cr0
                    else:
                        cr = inp.tile([128, 6, BLOCK], fp8, tag="cr")
                        nc.sync.dma_start(out=cr[:], in_=crt[b])
                    h1T = h1pool.tile([128, 4 * BLOCK], bf16, tag="h1T")
                    for m in range(4):
                        h1p = ph1.tile([128, BLOCK], f32, tag="h1p")
                        for p in range(3):
                            nc.tensor.matmul(
                                out=h1p[:],
                                lhsT=kw8_sb[:, m * 3 + p],
                                rhs=cr[:, 2 * p:2 * p + 2, :],
                                start=(p == 0),
                                stop=(p == 2),
                                perf_mode=DR,
                            )
                        if m < 2:
                            nc.scalar.activation(
                                out=h1T[:, m * BLOCK:(m + 1) * BLOCK],
                                in_=h1p[:],
                                func=Relu,
                                bias=b1_sb[:, m:m + 1],
                            )
                        else:
                            nc.vector.tensor_scalar(
                                out=h1T[:, m * BLOCK:(m + 1) * BLOCK],
                                in0=h1p[:],
                                scalar1=b1_sb[:, m:m + 1],
                                scalar2=0.0,
                                op0=ADD,
                                op1=MAX,
                            )
                    h1T_hist[b] = h1T

                # packed w3 group: strip j (col-tiled, partitions 32j:32j+2)
                # streams the bf16 h2T of block 4g+j; one f32 PSUM bank.
                # Emitted between the h1 and h2 stages so the group's ScalarE
                # evacuation unblocks ~850ns earlier, keeping ACT's queue from
                # delaying the next block's relu chain.
                if it in w3_at:
                    g = w3_at[it]
                    op = po.tile([128, BLOCK], f32, tag="op")
                    for j in range(WGRP):
                        h2T_prev = h2T_hist.pop(g * WGRP + j)
                        nc.tensor.matmul(
                            out=op[32 * j:32 * j + 2, :],
                            lhsT=w3_bf[:],
                            rhs=h2T_prev[:],
                            start=True,
                            stop=True,
                            tile_position=(0, 32 * j),
                        )
                    oac = opool.tile([128, BLOCK], f32, tag="oac")
                    nc.vector.tensor_scalar(
                        out=oac[:], in0=op[:],
                        scalar1=b3_sb[:, 0:1], scalar2=None, op0=ADD,
                    )
                    # out-DMA on the ScalarE HWDGE ring: keeps the Sync
                    # ring free for cr prefetches (PE was seen waiting on
                    # late cr DMAs behind queued output DMAs)
                    nc.scalar.dma_start(out=out_t[g], in_=oac[:])

                if 0 <= bh2 < n_blocks:
                    # h2 for block bh2 (its h1T finished a full iteration ago)
                    h1T_prev = h1T_hist.pop(bh2)
                    h2p = ph2.tile([128, BLOCK], f32, tag="h2p")
                    for k in range(4):
                        nc.tensor.matmul(
                            out=h2p[:],
                            lhsT=w2_sb[:, k * 128:(k + 1) * 128],
                            rhs=h1T_prev[:, k * BLOCK:(k + 1) * BLOCK],
                            start=(k == 0),
                            stop=(k == 3),
                        )
                    h2T = h2pool.tile([128, BLOCK], bf16, tag="h2T")
                    nc.scalar.activation(
                        out=h2T[:], in_=h2p[:], func=Relu, bias=b2_sb[:, 0:1],
                    )
                    h2T_hist[bh2] = h2T

    nc.compile()
    return nc


def _get_program():
    if "v3" not in _prog_cache:
        _prog_cache["v3"] = _build_program()
    return _prog_cache["v3"]


def kernel(emb, edge_index, W1, b1, W2, b2, W3, b3):
    if _REPO not in sys.path:
        sys.path.insert(0, _REPO)
    import ml_dtypes
    from concourse.bass_utils import run_bass_kernel_spmd

    BF = ml_dtypes.bfloat16
    E4 = ml_dtypes.float8_e4m3
    S = 32.0
    emb = np.ascontiguousarray(np.asarray(emb, dtype=np.float32))
    embT_f = np.ascontiguousarray(emb.T)           # [128, N_NODES] f32
    # fp8 hi/lo decomposition of the node table (x ~ hi + lo); xh/S is an
    # exact exponent shift, paired on-device with the S-scaled lo-weights
    xh_tab = embT_f.astype(E4)
    xl_tab = (embT_f - xh_tab.astype(np.float32)).astype(E4)
    xhs_tab = (xh_tab.astype(np.float32) / S).astype(E4)
    ei = np.asarray(edge_index)
    col = ei[0].astype(np.int64)
    row = ei[1].astype(np.int64)
    W1 = np.asarray(W1, np.float32)
    W2 = np.asarray(W2, np.float32)
    W3 = np.asarray(W3, np.float32)

    # w1 fp8 hi/lo DoubleRow packs; lo-weights scaled by S into e4m3's
    # normal range (subnormal-safe)
    whc = W1[0:128, :].astype(E4)
    wlc = ((W1[0:128, :] - whc.astype(np.float32)) * S).astype(E4)
    whr = W1[128:256, :].astype(E4)
    wlr = ((W1[128:256, :] - whr.astype(np.float32)) * S).astype(E4)
    kw8 = np.zeros((128, 12, 2, 128), E4)
    for m in range(4):
        sl = slice(m * 128, (m + 1) * 128)
        kw8[:, m * 3 + 0, 0] = whc[:, sl]
        kw8[:, m * 3 + 0, 1] = whc[:, sl]
        kw8[:, m * 3 + 1, 0] = whr[:, sl]
        kw8[:, m * 3 + 1, 1] = whr[:, sl]
        kw8[:, m * 3 + 2, 0] = wlc[:, sl]
        kw8[:, m * 3 + 2, 1] = wlr[:, sl]
    # packed bf16 constants: w2 | w3
    kw = np.zeros((128, 514), np.float32)
    for k in range(4):
        kw[:, k * 128:(k + 1) * 128] = W2[k * 128:(k + 1) * 128, :]
    kw[:, 512:514] = W3
    kw = kw.astype(BF)
    kb = np.zeros((128, 6), np.float32)
    kb[:, 0:4] = np.asarray(b1, np.float32).reshape(4, 128).T
    kb[:, 4] = np.asarray(b2, np.float32)
    b3f = np.asarray(b3, np.float32)
    for j in range(4):
        kb[32 * j:32 * j + 2, 5] = b3f

    in_maps = []
    for i in range(N_CORES):
        cpad = np.zeros(E_PAD, np.int64)
        rpad = np.zeros(E_PAD, np.int64)
        cpad[:E_SHARD] = col[i * E_SHARD:(i + 1) * E_SHARD]
        rpad[:E_SHARD] = row[i * E_SHARD:(i + 1) * E_SHARD]
        crt = np.empty((N_BLOCKS, 128, 6, BLOCK), E4)
        for s, (tab, idx) in enumerate(
            [(xh_tab, cpad), (xl_tab, cpad), (xh_tab, rpad), (xl_tab, rpad),
             (xhs_tab, cpad), (xhs_tab, rpad)]
        ):
            crt[:, :, s, :] = (
                tab[:, idx].reshape(128, N_BLOCKS, BLOCK).transpose(1, 0, 2)
            )
        in_maps.append({"crt": crt, "kw8": kw8, "kw": kw, "kb": kb})

    nc = _get_program()
    try:
        res = run_bass_kernel_spmd(nc, in_maps, list(range(N_CORES)), **RUN_KWARGS)
    except Exception:
        import ctypes

        lib = ctypes.CDLL("/opt/axon/libaxon_pjrt.so")
        lib.axon_reset.restype = ctypes.c_int64
        lib.axon_reset()
        res = run_bass_kernel_spmd(nc, in_maps, list(range(N_CORES)), **RUN_KWARGS)
    global LAST_RESULTS
    LAST_RESULTS = res

    out = np.empty((N_EDGES, 2), np.float32)
    for i in range(N_CORES):
        ot = res.results[i]["out_t"]  # [N_GRPS, 128, 512]
        # group g partitions 32j:32j+2 -> block 4g+j
        o4 = ot.reshape(N_GRPS, 4, 32, BLOCK)[:, :, 0:2, :]   # [G, 4, 2, 512]
        opad = o4.transpose(2, 0, 1, 3).reshape(2, E_PAD)
        out[i * E_SHARD:(i + 1) * E_SHARD] = opad[:, :E_SHARD].T
    return out


# revision 63
# speedup vs baseline: 1.3105x; 1.3105x over previous
"""Trainium2 Bass kernel for nn_ExtractorMLP: per-edge MLP over gathered node
embeddings, data-parallel over edges across 8 NeuronCores.

Per edge e: out = relu(relu(concat(emb[col[e]], emb[row[e]]) @ W1 + b1) @ W2 + b2) @ W3 + b3

v3 strategy ("host-sequenced gather, pure streaming MLP on device"):
The v1 kernel's critical path was the on-device gather: row-side indirect DMA
(784 GpSimd calls/core at ~1.2us) and col-side one-hot selection matmuls
(+2560 PE cycles/block).  v1 already shipped host-sequenced per-block chunk
data (chks/colf, ~100MB/core); v2+ pushes that to its logical end: the host
ships the gathered endpoint features directly, transposed to the [feature,
edge] layout the PE wants, interleaved per 512-edge block as crt[b] =
[colT_blk | rowT_blk] (same ~100MB/core of DRAM traffic).  The device is then
a pure streaming MLP at the PE roofline; the v2 trace showed 95.5% PE
occupancy with a 232ns MM issue period and <1us of total PE idle.

Refinements over the plain streaming version (measured 618us -> 547us):
- all-bf16 pipeline: with f32r operands every matmul paid ~20ns extra (no
  Fast Weight Load for fp32 -- LDWEIGHTS 187ns vs 97ns -- plus fp32-HIGH
  weight pairing overhead); bf16 operands hit the textbook warm issue period
  (~213ns = 512 cols @ 2.4GHz) and halve input DMA to ~50MB/core.  Measured
  596us (f32r) -> 548us (bf16); rel err 4.9e-3 vs 2e-2 gate.  fp8/DoubleRow
  cannot beat this: plain fp8 fails the gate (4.9e-2 emulated), and hi/lo
  compensation (lo-operands scaled by 32 past e4m3's subnormal cliff,
  rhs exponent-shifted to keep PSUM scale -- HW-validated at 4.0e-3) needs 3
  DR matmuls per m-group vs 2 plain ones; compensating both operands
  quadruples terms while DoubleRow only doubles throughput (measured 721us).
- w3 packing: the [128]->[2] output matmul wastes 126/128 PE rows.  Four
  consecutive blocks' w3 matmuls are issued back-to-back into disjoint
  32-column PE strips (tile_position=(0,32j), out partitions 32j:32j+2 of one
  PSUM bank) so they execute concurrently (~630ns per 4 blocks incl. the two
  tiling-mode-switch drains, vs 4x213ns unpacked), and a single [128,512]
  ACTIVATE evacuates all four (ACT cost is free-dim-based).  Host unpacks
  partition strips.  fp32 operands cannot column-tile (ISA dst-partition
  check); larger groups (7 waves per mode switch) measured WORSE (652us):
  wave N+3's PSUM bank depends on wave N's evacuation, which queues behind
  the regular relus in the ACT/DVE FIFOs.
- startup/tail: crt[0]+w1 DMAs issued first (first MM at ~10.5us); output
  DMAs ride the ScalarE HWDGE ring so cr prefetches never queue behind them
  on the Sync ring.
Final trace accounting: MM span 531us = 2352 MMs x 213ns + 49 w3 groups x
~630ns (within ~1us of the streaming-roofline model), plus ~10.5us fixed
startup and ~5.3us tail.  Measured 547us HW exec at full clock (the part
sometimes power-throttles to ~2.0GHz under sustained load -> ~650-715us);
rel err 4.9e-3.

Software pipelining keeps every engine's inputs at least one full block ahead
of use (PE never waits on relu evacuation): iteration i runs h1 pairs of
block i, h2 of block i-1, and the packed w3 group g=(i-5)/4 covering blocks
4g..4g+3.  PSUM: h1 m-groups rotate over 4 banks, h2 over 2, w3-out over 2.
Relu+bias evacuation is split between ScalarE (h1 m0/m1, out) and VectorE
(h1 m2/m3, h2; fused add-bias+max-0 tensor_scalar).  All matmuls in float32r
(TF32-like, ~3e-4 rel err, full PE rate); f32r DRAM tensors are DMAed
straight into f32r SBUF tiles (f32r is bit-identical to f32).  No sort, no
permutation: edges keep their natural order."""

import sys

import numpy as np

N_NODES = 50000
HIDDEN = 128
N_EDGES = 800000
N_CORES = 8
E_SHARD = N_EDGES // N_CORES

BLOCK = 512
N_BLOCKS = 196
E_PAD = N_BLOCKS * BLOCK   # 100352
WGRP = 4                   # blocks per packed w3 group (4 col-tiled strips)
N_GRPS = N_BLOCKS // WGRP  # 49

_REPO = "/opt/trn_rl_repo"
_prog_cache = {}
RUN_KWARGS = {}
LAST_RESULTS = None


def _build_program(n_blocks=N_BLOCKS, debug=False):
    if _REPO not in sys.path:
        sys.path.insert(0, _REPO)
    from concourse import bacc, mybir
    import concourse.tile as tile

    f32 = mybir.dt.float32
    f32r = mybir.dt.float32r
    bf16 = mybir.dt.bfloat16
    Relu = mybir.ActivationFunctionType.Relu
    Ident = mybir.ActivationFunctionType.Identity
    ADD = mybir.AluOpType.add
    MAX = mybir.AluOpType.max

    n_grps = n_blocks // WGRP

    nc = bacc.Bacc("TRN2", target_bir_lowering=False, debug=debug)
    # per-block gathered features: crt[b][:, 0:512] = emb[col].T for the
    # block's 512 edges, crt[b][:, 512:1024] = emb[row].T
    crt = nc.dram_tensor("crt", [n_blocks, 128, 2 * BLOCK], bf16, kind="ExternalInput")
    # packed constants: kw = [w1 (1024) | w2 (512) | w3 (2)] bf16,
    # kb = [b1t (4) | b2t (1) | b3r (1)] f32
    kw = nc.dram_tensor("kw", [128, 1538], bf16, kind="ExternalInput")
    kb = nc.dram_tensor("kb", [128, 6], f32, kind="ExternalInput")
    # packed output: group g holds blocks 4g..4g+3 at partitions 32j:32j+2
    out_t = nc.dram_tensor("out_t", [n_grps, 128, BLOCK], f32, kind="ExternalOutput")

    with tile.TileContext(nc) as tc:
        with (
            tc.tile_pool(name="const", bufs=1) as cp,
            tc.tile_pool(name="inp", bufs=6) as inp,
            tc.tile_pool(name="h1", bufs=3) as h1pool,
            tc.tile_pool(name="h2", bufs=8) as h2pool,
            tc.tile_pool(name="oac", bufs=3) as opool,
            tc.tile_pool(name="ps_h1", bufs=4, space="PSUM") as ph1,
            tc.tile_pool(name="ps_h2", bufs=2, space="PSUM") as ph2,
            tc.tile_pool(name="ps_o", bufs=2, space="PSUM") as po,
        ):
            # ---- persistent constants ----
            # crt[0] and w1 first: the startup-critical first h1 matmuls
            # gate only on these two transfers
            cr0 = inp.tile([128, 2 * BLOCK], bf16, tag="cr")
            nc.sync.dma_start(out=cr0[:], in_=crt[0])
            kw_sb = cp.tile([128, 1538], bf16)
            nc.sync.dma_start(out=kw_sb[:, 0:1024], in_=kw[:, 0:1024])
            kb_sb = cp.tile([128, 6], f32)
            nc.sync.dma_start(out=kb_sb[:], in_=kb[:])
            nc.sync.dma_start(out=kw_sb[:, 1024:1538], in_=kw[:, 1024:1538])
            w1_sb = kw_sb[:, 0:1024]
            w2_sb = kw_sb[:, 1024:1536]
            b1_sb = kb_sb[:, 0:4]
            b2_sb = kb_sb[:, 4:5]
            b3_sb = kb_sb[:, 5:6]
            w3_bf = kw_sb[:, 1536:1538]

            # w3 group schedule: full 8-block groups two iterations after the
            # group's last h2 stage; the 4-block tail group at the very end
            w3_at = {}
            for g in range(n_blocks // WGRP):
                w3_at[WGRP * g + WGRP + 2] = g
            if n_blocks % WGRP:
                w3_at[n_blocks + 2] = n_blocks // WGRP

            h1T_hist = {}   # block id -> h1T tile (consumed by h2 one iter later)
            h2T_hist = {}   # pair id -> paired h2T tile (consumed by w3 group)
            last_it = max(w3_at)
            for it in range(last_it + 1):
                b = it            # h1 stage block
                bh2 = it - 1      # h2 stage block

                if b < n_blocks:
                    if b == 0:
                        cr = cr0
                    else:
                        cr = inp.tile([128, 2 * BLOCK], bf16, tag="cr")
                        nc.sync.dma_start(out=cr[:], in_=crt[b])
                    h1T = h1pool.tile([128, 4 * BLOCK], bf16, tag="h1T")
                    for m in range(4):
                        h1p = ph1.tile([128, BLOCK], f32, tag="h1p")
                        nc.tensor.matmul(
                            out=h1p[:],
                            lhsT=w1_sb[:, m * 128:(m + 1) * 128],
                            rhs=cr[:, 0:BLOCK],
                            start=True,
                            stop=False,
                        )
                        nc.tensor.matmul(
                            out=h1p[:],
                            lhsT=w1_sb[:, 512 + m * 128:512 + (m + 1) * 128],
                            rhs=cr[:, BLOCK:2 * BLOCK],
                            start=False,
                            stop=True,
                        )
                        if m < 2:
                            nc.scalar.activation(
                                out=h1T[:, m * BLOCK:(m + 1) * BLOCK],
                                in_=h1p[:],
                                func=Relu,
                                bias=b1_sb[:, m:m + 1],
                            )
                        else:
                            nc.vector.tensor_scalar(
                                out=h1T[:, m * BLOCK:(m + 1) * BLOCK],
                                in0=h1p[:],
                                scalar1=b1_sb[:, m:m + 1],
                                scalar2=0.0,
                                op0=ADD,
                                op1=MAX,
                            )
                    h1T_hist[b] = h1T

                # packed w3 group: strip j (col-tiled, partitions 32j:32j+2)
                # streams the bf16 h2T of block 4g+j; one f32 PSUM bank.
                # Emitted between the h1 and h2 stages so the group's ScalarE
                # evacuation unblocks ~850ns earlier, keeping ACT's queue from
                # delaying the next block's relu chain.
                if it in w3_at:
                    g = w3_at[it]
                    op = po.tile([128, BLOCK], f32, tag="op")
                    for j in range(WGRP):
                        h2T_prev = h2T_hist.pop(g * WGRP + j)
                        nc.tensor.matmul(
                            out=op[32 * j:32 * j + 2, :],
                            lhsT=w3_bf[:],
                            rhs=h2T_prev[:],
                            start=True,
                            stop=True,
                            tile_position=(0, 32 * j),
                        )
                    oac = opool.tile([128, BLOCK], f32, tag="oac")
                    nc.scalar.activation(
                        out=oac[:], in_=op[:], func=Ident, bias=b3_sb[:, 0:1],
                    )
                    # out-DMA on the ScalarE HWDGE ring: keeps the Sync
                    # ring free for cr prefetches (PE was seen waiting on
                    # late cr DMAs behind queued output DMAs)
                    nc.scalar.dma_start(out=out_t[g], in_=oac[:])

                if 0 <= bh2 < n_blocks:
                    # h2 for block bh2 (its h1T finished a full iteration ago)
                    h1T_prev = h1T_hist.pop(bh2)
                    h2p = ph2.tile([128, BLOCK], f32, tag="h2p")
                    for k in range(4):
                        nc.tensor.matmul(
                            out=h2p[:],
                            lhsT=w2_sb[:, k * 128:(k + 1) * 128],
                            rhs=h1T_prev[:, k * BLOCK:(k + 1) * BLOCK],
                            start=(k == 0),
                            stop=(k == 3),
                        )
                    h2T = h2pool.tile([128, BLOCK], bf16, tag="h2T")
                    nc.vector.tensor_scalar(
                        out=h2T[:],
                        in0=h2p[:],
                        scalar1=b2_sb[:, 0:1],
                        scalar2=0.0,
                        op0=ADD,
                        op1=MAX,
                    )
                    h2T_hist[bh2] = h2T

    nc.compile()
    return nc


def _get_program():
    if "v3" not in _prog_cache:
        _prog_cache["v3"] = _build_program()
    return _prog_cache["v3"]


def kernel(emb, edge_index, W1, b1, W2, b2, W3, b3):
    if _REPO not in sys.path:
        sys.path.insert(0, _REPO)
    import ml_dtypes
    from concourse.bass_utils import run_bass_kernel_spmd

    BF = ml_dtypes.bfloat16
    emb = np.ascontiguousarray(np.asarray(emb, dtype=np.float32))
    embT = np.ascontiguousarray(emb.T.astype(BF))  # [128, N_NODES] bf16
    ei = np.asarray(edge_index)
    col = ei[0].astype(np.int64)
    row = ei[1].astype(np.int64)
    W1 = np.asarray(W1, np.float32)
    W2 = np.asarray(W2, np.float32)
    W3 = np.asarray(W3, np.float32)

    # packed constants
    kw = np.zeros((128, 1538), np.float32)
    kw[:, 0:512] = W1[0:128, :]
    kw[:, 512:1024] = W1[128:256, :]
    for k in range(4):
        kw[:, 1024 + k * 128:1024 + (k + 1) * 128] = W2[k * 128:(k + 1) * 128, :]
    kw[:, 1536:1538] = W3
    kw = kw.astype(BF)
    kb = np.zeros((128, 6), np.float32)
    kb[:, 0:4] = np.asarray(b1, np.float32).reshape(4, 128).T
    kb[:, 4] = np.asarray(b2, np.float32)
    b3f = np.asarray(b3, np.float32)
    for j in range(4):
        kb[32 * j:32 * j + 2, 5] = b3f

    in_maps = []
    for i in range(N_CORES):
        cpad = np.zeros(E_PAD, np.int64)
        rpad = np.zeros(E_PAD, np.int64)
        cpad[:E_SHARD] = col[i * E_SHARD:(i + 1) * E_SHARD]
        rpad[:E_SHARD] = row[i * E_SHARD:(i + 1) * E_SHARD]
        crt = np.empty((N_BLOCKS, 128, 2 * BLOCK), BF)
        crt[:, :, 0:BLOCK] = (
            embT[:, cpad].reshape(128, N_BLOCKS, BLOCK).transpose(1, 0, 2)
        )
        crt[:, :, BLOCK:2 * BLOCK] = (
            embT[:, rpad].reshape(128, N_BLOCKS, BLOCK).transpose(1, 0, 2)
        )
        in_maps.append({"crt": crt, "kw": kw, "kb": kb})

    nc = _get_program()
    try:
        res = run_bass_kernel_spmd(nc, in_maps, list(range(N_CORES)), **RUN_KWARGS)
    except Exception:
        import ctypes

        lib = ctypes.CDLL("/opt/axon/libaxon_pjrt.so")
        lib.axon_reset.restype = ctypes.c_int64
        lib.axon_reset()
        res = run_bass_kernel_spmd(nc, in_maps, list(range(N_CORES)), **RUN_KWARGS)
    global LAST_RESULTS
    LAST_RESULTS = res

    out = np.empty((N_EDGES, 2), np.float32)
    for i in range(N_CORES):
        ot = res.results[i]["out_t"]  # [N_GRPS, 128, 512]
        # group g partitions 32j:32j+2 -> block 4g+j
        o4 = ot.reshape(N_GRPS, 4, 32, BLOCK)[:, :, 0:2, :]   # [G, 4, 2, 512]
        opad = o4.transpose(2, 0, 1, 3).reshape(2, E_PAD)
        out[i * E_SHARD:(i + 1) * E_SHARD] = opad[:, :E_SHARD].T
    return out
